# revision 8
# baseline (speedup 1.0000x reference)
"""Multi-head attention + residual + LayerNorm Bass kernel for Trainium2.

Shards across 8 NeuronCores as (batch, query-half): core c handles
query rows [ (c%2)*1024 : (c%2+1)*1024 ] of batch c//2.  Each core is
fully independent (K/V projections for its batch are recomputed on both
cores sharing the batch; no collectives).

Per-core device pipeline:
  1. q^T, k^T (bf16) and v (natural layout, bf16, with a ones column per
     head) via fp32r projections from host-transposed activations.
  2. Per head: scores^T[k,q] on PE; exp on ScalarE (PSUM->SBUF, bf16);
     0/1-mask multiply split DVE/GPSIMD; context matmul accumulates
     (v|1)^T @ em^T so row 64 of the context PSUM is the softmax
     denominator.
  3. PE tile-transposes em^T back to [q,k]; the PSUM eviction applies the
     per-row 1/sum (ScalarE activation scale-AP or DVE tensor_scalar) and
     the result is DMA'd out as the attention-probability output.
  4. Context rows are transposed/normalized the same way, then the output
     projection runs with swapped operands to directly produce [q, d]
     layout; residual + LayerNorm on DVE/ScalarE; y DMA'd out.
"""

import numpy as np
import ml_dtypes

D_MODEL = 1024
N_HEADS = 16
D_K = D_MODEL // N_HEADS
B, S = 4, 2048
P = 128

_PROGRAM_CACHE = {}


def _build_program(SQ, SK, D, H):
    """Build + compile the single-core Bass program (same program runs
    SPMD on all cores with different data)."""
    import concourse.bass as bass
    import concourse.mybir as mybir
    import concourse.tile as tile
    from concourse import bacc
    from concourse.masks import make_identity
    from contextlib import ExitStack

    F32 = mybir.dt.float32
    F32R = mybir.dt.float32r
    BF16 = mybir.dt.bfloat16
    AF = mybir.ActivationFunctionType

    DK = D // H
    DCH = D // P          # d_model chunks of 128
    KC = SK // P          # key chunks of 128
    QN = SQ // P          # query chunks of 128
    QBLK = 512 if SQ % 512 == 0 else SQ
    NQB = SQ // QBLK      # query blocks (512 wide)
    QCC = QBLK // P       # 128-q tiles per block
    KGRP = 4 if KC % 4 == 0 else KC   # key chunks per attn-evict group
    NKG = KC // KGRP
    HPC = P // DK         # heads per 128-partition chunk (2)
    SB = min(512, SQ)     # projection free-dim block
    VB = min(512, D)      # v / out-proj dout block

    nc = bacc.Bacc("TRN2", target_bir_lowering=False, debug=False, num_devices=8)

    # ---- DRAM I/O ----
    xqT = nc.dram_tensor("xqT", [D, SQ], F32R, kind="ExternalInput")
    xkT = nc.dram_tensor("xkT", [D, SK], F32R, kind="ExternalInput")
    xvT = nc.dram_tensor("xvT", [D, SK], F32R, kind="ExternalInput")
    xq_res = nc.dram_tensor("xq_res", [SQ, D], F32, kind="ExternalInput")
    maskT = nc.dram_tensor("maskT", [SK, SQ], BF16, kind="ExternalInput")
    wqT = nc.dram_tensor("wqT", [D, D], F32R, kind="ExternalInput")
    wkT = nc.dram_tensor("wkT", [D, D], F32R, kind="ExternalInput")
    wvT = nc.dram_tensor("wvT", [D, D], F32R, kind="ExternalInput")
    woT = nc.dram_tensor("woT", [D, D], F32, kind="ExternalInput")
    bq = nc.dram_tensor("bq", [D], F32, kind="ExternalInput")
    bk = nc.dram_tensor("bk", [D], F32, kind="ExternalInput")
    bv = nc.dram_tensor("bv", [D], F32, kind="ExternalInput")
    bo = nc.dram_tensor("bo", [D], F32, kind="ExternalInput")
    ln_g = nc.dram_tensor("ln_g", [D], F32, kind="ExternalInput")
    ln_b = nc.dram_tensor("ln_b", [D], F32, kind="ExternalInput")
    attn_out = nc.dram_tensor("attn", [H, SQ, SK], F32, kind="ExternalOutput")
    y_out = nc.dram_tensor("y", [SQ, D], F32, kind="ExternalOutput")

    def bcast(dram_ap, parts=P):
        # [D] DRAM vector -> [parts, D] partition-broadcast AP
        return bass.AP(
            tensor=dram_ap.tensor,
            offset=dram_ap.offset,
            ap=[[0, parts]] + [list(x) for x in dram_ap.ap],
        )

    with tile.TileContext(nc) as tc:
        with ExitStack() as ctx:
            # ---- persistent pools ----
            const = ctx.enter_context(tc.tile_pool(name="const", bufs=1))
            persist = ctx.enter_context(tc.tile_pool(name="persist", bufs=1))

            id_bf = const.tile([P, P], BF16)
            make_identity(nc, id_bf)
            id_f32 = const.tile([P, P], F32)
            make_identity(nc, id_f32)
            ones11 = const.tile([1, 1], F32)
            nc.vector.memset(ones11, 1.0)

            # per-partition bias columns for q/k eviction ([128, DCH])
            bq_sb = const.tile([P, DCH], F32)
            nc.sync.dma_start(bq_sb, bq.ap().rearrange("(c p) -> p c", p=P))
            bk_sb = const.tile([P, DCH], F32)
            nc.sync.dma_start(bk_sb, bk.ap().rearrange("(c p) -> p c", p=P))
            # free-dim broadcast rows for v bias
            bv_bc = const.tile([P, D], F32)
            nc.sync.dma_start(bv_bc, bcast(bv.ap()))

            qT_sb = persist.tile([P, DCH, SQ], BF16)      # q^T  [d, q]
            kT_sb = persist.tile([P, DCH, SK], BF16)      # k^T  [d, k]
            vaug_sb = persist.tile([P, KC, H * (DK + 1)], BF16)  # v | ones
            c_nat = persist.tile([P, QN, D], BF16)        # normalized context [q, d]

            nc.vector.memset(vaug_sb, 1.0)  # ones columns survive the evicts

            # ================= Phase 1: projections =================
            with tc.tile_pool(name="wpool", bufs=1) as wpool, \
                 tc.tile_pool(name="xpool", bufs=3) as xpool:

              with tc.tile_pool(name="p1psum", bufs=1, space="PSUM") as p1psum:
                # ---- q^T and k^T : [dout part, seq free] ----
                for (w_dram, x_dram, dest, bias_sb, SLEN) in (
                    (wqT, xqT, qT_sb, bq_sb, SQ),
                    (wkT, xkT, kT_sb, bk_sb, SK),
                ):
                    w_sb = wpool.tile([P, DCH, D], F32R, tag="w")
                    nc.sync.dma_start(
                        w_sb, w_dram.ap().rearrange("(c p) n -> p c n", p=P)
                    )
                    for sc in range(SLEN // SB):
                        xch = xpool.tile([P, DCH, SB], F32R, tag="x")
                        nc.sync.dma_start(
                            xch,
                            x_dram.ap()[:, sc * SB : sc * SB + SB].rearrange(
                                "(c p) s -> p c s", p=P
                            ),
                        )
                        psums = [
                            p1psum.tile([P, SB], F32, tag=f"pp{dco}", name=f"pp{dco}")
                            for dco in range(DCH)
                        ]
                        for dci in range(DCH):
                            for dco in range(DCH):
                                nc.tensor.matmul(
                                    psums[dco],
                                    lhsT=w_sb[:, dci, dco * P : dco * P + P],
                                    rhs=xch[:, dci, :],
                                    start=(dci == 0),
                                    stop=(dci == DCH - 1),
                                )
                        for dco in range(DCH):
                            nc.vector.tensor_scalar_add(
                                dest[:, dco, sc * SB : sc * SB + SB],
                                psums[dco],
                                bias_sb[:, dco : dco + 1],
                            )

              # ---- v natural layout [s part, dout free], head-interleaved ----
              with tc.tile_pool(name="vpsum", bufs=2, space="PSUM") as vpsum:
                w_sb = wpool.tile([P, DCH, D], F32R, tag="w")
                nc.sync.dma_start(w_sb, wvT.ap().rearrange("(c p) n -> p c n", p=P))
                for sc in range(KC):
                    xch = xpool.tile([P, DCH, P], F32R, tag="xv")
                    nc.sync.dma_start(
                        xch,
                        xvT.ap()[:, sc * P : sc * P + P].rearrange(
                            "(c p) s -> p c s", p=P
                        ),
                    )
                    for nh in range(D // VB):
                        pv = vpsum.tile([P, VB], F32, tag="pv")
                        for dci in range(DCH):
                            nc.tensor.matmul(
                                pv,
                                lhsT=xch[:, dci, :],
                                rhs=w_sb[:, dci, nh * VB : nh * VB + VB],
                                start=(dci == 0),
                                stop=(dci == DCH - 1),
                            )
                        h0 = nh * (VB // DK)
                        dest = vaug_sb[:, sc, :].rearrange(
                            "p (h e) -> p h e", e=DK + 1
                        )[:, h0 : h0 + VB // DK, 0:DK]
                        nc.vector.tensor_tensor(
                            dest,
                            pv.rearrange("p (h e) -> p h e", e=DK),
                            bv_bc[:, nh * VB : nh * VB + VB].rearrange(
                                "p (h e) -> p h e", e=DK
                            ),
                            mybir.AluOpType.add,
                        )

            # ================= Phase 2: attention =================
            with tc.tile_pool(name="mpool", bufs=1) as mpool, \
                 tc.tile_pool(name="empool", bufs=2) as empool, \
                 tc.tile_pool(name="stpool", bufs=2) as stpool, \
                 tc.tile_pool(name="smpool", bufs=3) as smpool, \
                 tc.tile_pool(name="spsum", bufs=2, space="PSUM") as spsum, \
                 tc.tile_pool(name="cpsum", bufs=2, space="PSUM") as cpsum, \
                 tc.tile_pool(name="tpsum", bufs=2, space="PSUM") as tpsum, \
                 tc.tile_pool(name="rpsum", bufs=1, space="PSUM") as rpsum, \
                 tc.tile_pool(name="cnpsum", bufs=1, space="PSUM") as cnpsum:

                maskT_sb = mpool.tile([P, KC, SQ], BF16)
                nc.sync.dma_start(
                    maskT_sb, maskT.ap().rearrange("(c p) q -> p c q", p=P)
                )

                for qb in range(NQB):
                    q0 = qb * QBLK
                    for h in range(H):
                        pr0 = (h % HPC) * DK     # partition offset within d-chunk
                        dch = h // HPC           # which d-chunk holds this head
                        em = empool.tile([P, KC, QBLK], BF16, tag="em")
                        pc = cpsum.tile([DK + 1, QBLK], F32, tag="pc")
                        for kc in range(KC):
                            ps = spsum.tile([P, QBLK], F32, tag="ps")
                            nc.tensor.matmul(
                                ps,
                                lhsT=kT_sb[pr0 : pr0 + DK, dch, kc * P : kc * P + P],
                                rhs=qT_sb[pr0 : pr0 + DK, dch, q0 : q0 + QBLK],
                            )
                            nc.scalar.activation(
                                em[:, kc, :], ps, AF.Exp, scale=1.0 / np.sqrt(DK)
                            )
                            m_sl = maskT_sb[:, kc, q0 : q0 + QBLK]
                            if kc % 2 == 0:
                                nc.gpsimd.tensor_mul(em[:, kc, :], em[:, kc, :], m_sl)
                            else:
                                nc.vector.tensor_mul(em[:, kc, :], em[:, kc, :], m_sl)
                            nc.tensor.matmul(
                                pc,
                                lhsT=vaug_sb[:, kc, h * (DK + 1) : (h + 1) * (DK + 1)],
                                rhs=em[:, kc, :],
                                start=(kc == 0),
                                stop=(kc == KC - 1),
                            )
                        # softmax denominators -> per-q-column reciprocals
                        srow = smpool.tile([1, QBLK], F32, tag="srow")
                        nc.scalar.copy(srow, pc[DK : DK + 1, :])
                        prc = rpsum.tile([P, QCC], F32, tag="prc")
                        for j in range(QCC):
                            nc.tensor.matmul(
                                prc[:, j : j + 1],
                                lhsT=srow[0:1, j * P : j * P + P],
                                rhs=ones11,
                            )
                        recip = smpool.tile([P, QCC], F32, tag="recip")
                        nc.vector.reciprocal(recip, prc)

                        # context rows: transpose + normalize -> c_nat
                        crow = smpool.tile([DK, QBLK], F32, tag="crow")
                        nc.scalar.copy(crow, pc[0:DK, :])
                        pcn = cnpsum.tile([P, QCC * DK], F32, tag="pcn")
                        for j in range(QCC):
                            nc.tensor.transpose(
                                pcn[:, j * DK : (j + 1) * DK],
                                crow[:, j * P : j * P + P],
                                id_f32[:DK, :DK],
                            )
                        for j in range(QCC):
                            nc.scalar.activation(
                                c_nat[:, qb * QCC + j, h * DK : (h + 1) * DK],
                                pcn[:, j * DK : (j + 1) * DK],
                                AF.Copy,
                                scale=recip[:, j : j + 1],
                            )

                        # attention probabilities: transpose, normalize, DMA out
                        for j in range(QCC):
                            stage = stpool.tile([P, SK], F32, tag="stage")
                            for kg in range(NKG):
                                pt = tpsum.tile([P, KGRP * P], BF16, tag="pt")
                                for kk in range(KGRP):
                                    kc = kg * KGRP + kk
                                    nc.tensor.transpose(
                                        pt[:, kk * P : kk * P + P],
                                        em[:, kc, j * P : j * P + P],
                                        id_bf,
                                    )
                                dst = stage[:, kg * KGRP * P : (kg + 1) * KGRP * P]
                                if kg % 4 == 0:
                                    nc.scalar.activation(
                                        dst, pt, AF.Copy, scale=recip[:, j : j + 1]
                                    )
                                else:
                                    nc.vector.tensor_scalar_mul(
                                        dst, pt, recip[:, j : j + 1]
                                    )
                            nc.sync.dma_start(
                                attn_out.ap()[h, (qb * QCC + j) * P : (qb * QCC + j + 1) * P, :],
                                stage,
                            )

            # ================= Phase 3: out proj + residual + LN =================
            with tc.tile_pool(name="wopool", bufs=1) as wopool, \
                 tc.tile_pool(name="bcpool", bufs=1) as bcpool, \
                 tc.tile_pool(name="opool", bufs=2) as opool, \
                 tc.tile_pool(name="lnpool", bufs=2) as lnpool, \
                 tc.tile_pool(name="p3psum", bufs=2, space="PSUM") as p3psum, \
                 tc.tile_pool(name="ctpsum", bufs=2, space="PSUM") as ctpsum:

                # cT: [d part, q free] from c_nat via PE transposes
                cT_sb = wopool.tile([P, DCH, SQ], BF16, tag="cT")
                DG = 4 if DCH % 4 == 0 else DCH
                for qn in range(QN):
                    for dcg in range(DCH // DG):
                        pw = ctpsum.tile([P, DG * P], BF16, tag="pw")
                        for dd in range(DG):
                            dc = dcg * DG + dd
                            nc.tensor.transpose(
                                pw[:, dd * P : dd * P + P],
                                c_nat[:, qn, dc * P : dc * P + P],
                                id_bf,
                            )
                        nc.vector.tensor_copy(
                            cT_sb[:, dcg * DG : dcg * DG + DG, qn * P : qn * P + P],
                            pw.rearrange("p (c q) -> p c q", q=P),
                        )

                # Wo in bf16 (stream the f32 chunks through a small pool)
                wo_bf = wopool.tile([P, DCH, D], BF16, tag="wobf")
                for dc in range(DCH):
                    wo_f32 = bcpool.tile([P, D], F32, tag="wof")
                    nc.sync.dma_start(
                        wo_f32, woT.ap()[dc * P : dc * P + P, :]
                    )
                    nc.vector.tensor_copy(wo_bf[:, dc, :], wo_f32)

                bo_bc = wopool.tile([P, D], F32, tag="bo")
                nc.sync.dma_start(bo_bc, bcast(bo.ap()))
                g_bc = wopool.tile([P, D], F32, tag="g")
                nc.sync.dma_start(g_bc, bcast(ln_g.ap()))
                b_bc = wopool.tile([P, D], F32, tag="b")
                nc.sync.dma_start(b_bc, bcast(ln_b.ap()))

                for qn in range(QN):
                    onat = opool.tile([P, D], F32, tag="onat")
                    for nh in range(D // VB):
                        po = p3psum.tile([P, VB], F32, tag="po")
                        for dci in range(DCH):
                            nc.tensor.matmul(
                                po,
                                lhsT=cT_sb[:, dci, qn * P : qn * P + P],
                                rhs=wo_bf[:, dci, nh * VB : nh * VB + VB],
                                start=(dci == 0),
                                stop=(dci == DCH - 1),
                            )
                        nc.vector.tensor_tensor(
                            onat[:, nh * VB : nh * VB + VB],
                            po,
                            bo_bc[:, nh * VB : nh * VB + VB],
                            mybir.AluOpType.add,
                        )
                    xr = opool.tile([P, D], F32, tag="xr")
                    nc.sync.dma_start(xr, xq_res.ap()[qn * P : qn * P + P, :])
                    nc.vector.tensor_add(onat, onat, xr)

                    # LayerNorm
                    nmean = lnpool.tile([P, 1], F32, tag="nmean")
                    nc.vector.reduce_sum(nmean, onat, axis=mybir.AxisListType.X)
                    nc.vector.tensor_scalar_mul(nmean, nmean, -1.0 / D)
                    xc = lnpool.tile([P, D], F32, tag="xc")
                    nc.vector.tensor_scalar_add(xc, onat, nmean[:, 0:1])
                    vs = lnpool.tile([P, 1], F32, tag="vs")
                    nc.scalar.activation(onat, xc, AF.Square, accum_out=vs)
                    nc.vector.tensor_scalar_mul(vs, vs, 1.0 / D)
                    nc.vector.tensor_scalar_add(vs, vs, 1e-5)
                    st = lnpool.tile([P, 1], F32, tag="st")
                    nc.scalar.activation(st, vs, AF.Sqrt)
                    rstd = lnpool.tile([P, 1], F32, tag="rstd")
                    nc.vector.reciprocal(rstd, st)
                    nc.vector.tensor_scalar_mul(xc, xc, rstd[:, 0:1])
                    nc.vector.tensor_mul(xc, xc, g_bc)
                    nc.vector.tensor_add(xc, xc, b_bc)
                    nc.sync.dma_start(y_out.ap()[qn * P : qn * P + P, :], xc)

    nc.compile()
    return nc


def _get_program(SQ, SK, D, H):
    key = (SQ, SK, D, H)
    if key not in _PROGRAM_CACHE:
        _PROGRAM_CACHE[key] = _build_program(SQ, SK, D, H)
    return _PROGRAM_CACHE[key]


def _make_in_maps(Q, K, V, attn_mask, Wq, bq, Wk, bk, Wv, bv, Wo, bo, ln_g, ln_b,
                  n_cores=8):
    f32 = np.float32
    bf16 = ml_dtypes.bfloat16
    SQ = Q.shape[1] * Q.shape[0] // n_cores  # rows per core
    per_batch = n_cores // Q.shape[0]

    wqT = np.ascontiguousarray(np.asarray(Wq, f32).T)
    wkT = np.ascontiguousarray(np.asarray(Wk, f32).T)
    wvT = np.ascontiguousarray(np.asarray(Wv, f32).T)
    woT = np.ascontiguousarray(np.asarray(Wo, f32).T)
    shared = {
        "wqT": wqT, "wkT": wkT, "wvT": wvT, "woT": woT,
        "bq": np.asarray(bq, f32), "bk": np.asarray(bk, f32),
        "bv": np.asarray(bv, f32), "bo": np.asarray(bo, f32),
        "ln_g": np.asarray(ln_g, f32), "ln_b": np.asarray(ln_b, f32),
    }

    in_maps = []
    kT_cache = {}
    for c in range(n_cores):
        b, sh = c // per_batch, c % per_batch
        qsl = np.asarray(Q[b, sh * SQ : (sh + 1) * SQ], f32)
        if b not in kT_cache:
            kT_cache[b] = (
                np.ascontiguousarray(np.asarray(K[b], f32).T),
                np.ascontiguousarray(np.asarray(V[b], f32).T),
            )
        xkT, xvT = kT_cache[b]
        m = np.asarray(attn_mask[b, sh * SQ : (sh + 1) * SQ])  # [SQ, SK] int
        maskT = np.ascontiguousarray(m.T).astype(bf16)
        in_maps.append({
            "xqT": np.ascontiguousarray(qsl.T),
            "xkT": xkT,
            "xvT": xvT,
            "xq_res": qsl,
            "maskT": maskT,
            **shared,
        })
    return in_maps


def kernel(Q, K, V, attn_mask, Wq, bq, Wk, bk, Wv, bv, Wo, bo, ln_g, ln_b):
    from concourse.bass_utils import run_bass_kernel_spmd

    Q = np.asarray(Q)
    Bsz, Ssz, Dsz = Q.shape
    n_cores = 8
    per_batch = n_cores // Bsz
    SQ = Ssz // per_batch
    H = N_HEADS

    nc = _get_program(SQ, Ssz, Dsz, H)
    in_maps = _make_in_maps(Q, K, V, attn_mask, Wq, bq, Wk, bk, Wv, bv,
                            Wo, bo, ln_g, ln_b, n_cores)
    res = run_bass_kernel_spmd(nc, in_maps, list(range(n_cores)))

    y_full = np.empty((Bsz, Ssz, Dsz), np.float32)
    attn_full = np.empty((Bsz, H, Ssz, Ssz), np.float32)
    for c, r in enumerate(res.results):
        b, sh = c // per_batch, c % per_batch
        y_full[b, sh * SQ : (sh + 1) * SQ] = r["y"]
        attn_full[b, :, sh * SQ : (sh + 1) * SQ, :] = r["attn"]
    return (y_full, attn_full)


# revision 10
# speedup vs baseline: 1.2020x; 1.2020x over previous
"""Multi-head attention + residual + LayerNorm Bass kernel for Trainium2.

Shards across 8 NeuronCores as (batch, query-half): core c handles
query rows [ (c%2)*1024 : (c%2+1)*1024 ] of batch c//2.  Each core is
fully independent (K/V projections for its batch are recomputed on both
cores sharing the batch; no collectives).

Per-core device pipeline:
  1. q^T, k^T (bf16) and v (natural layout, bf16, with a ones column per
     head) via fp32r projections from host-transposed activations.
  2. Per head: scores^T[k,q] on PE; exp on ScalarE (PSUM->SBUF, bf16);
     0/1-mask multiply split DVE/GPSIMD; context matmul accumulates
     (v|1)^T @ em^T so row 64 of the context PSUM is the softmax
     denominator.
  3. PE tile-transposes em^T back to [q,k]; the PSUM eviction applies the
     per-row 1/sum (ScalarE activation scale-AP or DVE tensor_scalar) and
     the result is DMA'd out as the attention-probability output.
  4. Context rows are transposed/normalized the same way, then the output
     projection runs with swapped operands to directly produce [q, d]
     layout; residual + LayerNorm on DVE/ScalarE; y DMA'd out.
"""

import numpy as np
import ml_dtypes

D_MODEL = 1024
N_HEADS = 16
D_K = D_MODEL // N_HEADS
B, S = 4, 2048
P = 128

_PROGRAM_CACHE = {}


def _build_program(SQ, SK, D, H):
    """Build + compile the single-core Bass program (same program runs
    SPMD on all cores with different data)."""
    import concourse.bass as bass
    import concourse.mybir as mybir
    import concourse.tile as tile
    from concourse import bacc
    from concourse.masks import make_identity
    from contextlib import ExitStack

    F32 = mybir.dt.float32
    F32R = mybir.dt.float32r
    BF16 = mybir.dt.bfloat16
    AF = mybir.ActivationFunctionType

    DK = D // H
    DCH = D // P          # d_model chunks of 128
    KC = SK // P          # key chunks of 128
    QN = SQ // P          # query chunks of 128
    QBLK = 512 if SQ % 512 == 0 else SQ
    NQB = SQ // QBLK      # query blocks (512 wide)
    QCC = QBLK // P       # 128-q tiles per block
    KGRP = 4 if KC % 4 == 0 else KC   # key chunks per attn-evict group
    NKG = KC // KGRP
    HPC = P // DK         # heads per 128-partition chunk (2)
    SB = min(512, SQ)     # projection free-dim block
    VB = min(512, D)      # v / out-proj dout block

    nc = bacc.Bacc("TRN2", target_bir_lowering=False, debug=False, num_devices=8)

    # ---- DRAM I/O ----
    xqT = nc.dram_tensor("xqT", [D, SQ], BF16, kind="ExternalInput")
    xkT = nc.dram_tensor("xkT", [D, SK], BF16, kind="ExternalInput")
    xvT = nc.dram_tensor("xvT", [D, SK], BF16, kind="ExternalInput")
    xq_res = nc.dram_tensor("xq_res", [SQ, D], F32, kind="ExternalInput")
    maskT = nc.dram_tensor("maskT", [SK, SQ], BF16, kind="ExternalInput")
    wqT = nc.dram_tensor("wqT", [D, D], BF16, kind="ExternalInput")
    wkT = nc.dram_tensor("wkT", [D, D], BF16, kind="ExternalInput")
    wvT = nc.dram_tensor("wvT", [D, D], BF16, kind="ExternalInput")
    woT = nc.dram_tensor("woT", [D, D], BF16, kind="ExternalInput")
    bq = nc.dram_tensor("bq", [D], F32, kind="ExternalInput")
    bk = nc.dram_tensor("bk", [D], F32, kind="ExternalInput")
    bv = nc.dram_tensor("bv", [D], F32, kind="ExternalInput")
    bo = nc.dram_tensor("bo", [D], F32, kind="ExternalInput")
    ln_g = nc.dram_tensor("ln_g", [D], F32, kind="ExternalInput")
    ln_b = nc.dram_tensor("ln_b", [D], F32, kind="ExternalInput")
    attn_out = nc.dram_tensor("attn", [H, SQ, SK], F32, kind="ExternalOutput")
    y_out = nc.dram_tensor("y", [SQ, D], F32, kind="ExternalOutput")

    def bcast(dram_ap, parts=P):
        # [D] DRAM vector -> [parts, D] partition-broadcast AP
        return bass.AP(
            tensor=dram_ap.tensor,
            offset=dram_ap.offset,
            ap=[[0, parts]] + [list(x) for x in dram_ap.ap],
        )

    with tile.TileContext(nc) as tc:
        with ExitStack() as ctx:
            # ---- persistent pools ----
            const = ctx.enter_context(tc.tile_pool(name="const", bufs=1))
            persist = ctx.enter_context(tc.tile_pool(name="persist", bufs=1))

            id_bf = const.tile([P, P], BF16)
            make_identity(nc, id_bf)
            id_f32 = const.tile([P, P], F32)
            make_identity(nc, id_f32)
            ones11 = const.tile([1, 1], F32)
            nc.vector.memset(ones11, 1.0)

            # per-partition bias columns for q/k eviction ([128, DCH])
            bq_sb = const.tile([P, DCH], F32)
            nc.sync.dma_start(bq_sb, bq.ap().rearrange("(c p) -> p c", p=P))
            bk_sb = const.tile([P, DCH], F32)
            nc.sync.dma_start(bk_sb, bk.ap().rearrange("(c p) -> p c", p=P))
            # free-dim broadcast rows for v bias
            bv_bc = const.tile([P, D], F32)
            nc.sync.dma_start(bv_bc, bcast(bv.ap()))

            qT_sb = persist.tile([P, DCH, SQ], BF16)      # q^T  [d, q]
            kT_sb = persist.tile([P, DCH, SK], BF16)      # k^T  [d, k]
            vaug_sb = persist.tile([P, KC, H * (DK + 1)], BF16)  # v | ones
            c_nat = persist.tile([P, QN, D], BF16)        # normalized context [q, d]

            nc.gpsimd.memset(vaug_sb, 1.0)  # ones columns survive the evicts

            # ================= Phase 1: projections =================
            with tc.tile_pool(name="wpool", bufs=1) as wpool, \
                 tc.tile_pool(name="xpool", bufs=3) as xpool:

              with tc.tile_pool(name="p1psum", bufs=1, space="PSUM") as p1psum:
                # ---- q^T and k^T : [dout part, seq free] ----
                for (w_dram, x_dram, dest, bias_sb, SLEN) in (
                    (wqT, xqT, qT_sb, bq_sb, SQ),
                    (wkT, xkT, kT_sb, bk_sb, SK),
                ):
                    w_sb = wpool.tile([P, DCH, D], BF16, tag="w")
                    nc.sync.dma_start(
                        w_sb, w_dram.ap().rearrange("(c p) n -> p c n", p=P)
                    )
                    for sc in range(SLEN // SB):
                        xch = xpool.tile([P, DCH, SB], BF16, tag="x")
                        nc.sync.dma_start(
                            xch,
                            x_dram.ap()[:, sc * SB : sc * SB + SB].rearrange(
                                "(c p) s -> p c s", p=P
                            ),
                        )
                        psums = [
                            p1psum.tile([P, SB], F32, tag=f"pp{dco}", name=f"pp{dco}")
                            for dco in range(DCH)
                        ]
                        for dci in range(DCH):
                            for dco in range(DCH):
                                nc.tensor.matmul(
                                    psums[dco],
                                    lhsT=w_sb[:, dci, dco * P : dco * P + P],
                                    rhs=xch[:, dci, :],
                                    start=(dci == 0),
                                    stop=(dci == DCH - 1),
                                )
                        for dco in range(DCH):
                            nc.vector.tensor_scalar_add(
                                dest[:, dco, sc * SB : sc * SB + SB],
                                psums[dco],
                                bias_sb[:, dco : dco + 1],
                            )

              # ---- v natural layout [s part, dout free], head-interleaved ----
              with tc.tile_pool(name="vpsum", bufs=2, space="PSUM") as vpsum:
                w_sb = wpool.tile([P, DCH, D], BF16, tag="w")
                nc.sync.dma_start(w_sb, wvT.ap().rearrange("(c p) n -> p c n", p=P))
                for sc in range(KC):
                    xch = xpool.tile([P, DCH, P], BF16, tag="xv")
                    nc.sync.dma_start(
                        xch,
                        xvT.ap()[:, sc * P : sc * P + P].rearrange(
                            "(c p) s -> p c s", p=P
                        ),
                    )
                    for nh in range(D // VB):
                        pv = vpsum.tile([P, VB], F32, tag="pv")
                        for dci in range(DCH):
                            nc.tensor.matmul(
                                pv,
                                lhsT=xch[:, dci, :],
                                rhs=w_sb[:, dci, nh * VB : nh * VB + VB],
                                start=(dci == 0),
                                stop=(dci == DCH - 1),
                            )
                        h0 = nh * (VB // DK)
                        dest = vaug_sb[:, sc, :].rearrange(
                            "p (h e) -> p h e", e=DK + 1
                        )[:, h0 : h0 + VB // DK, 0:DK]
                        nc.vector.tensor_tensor(
                            dest,
                            pv.rearrange("p (h e) -> p h e", e=DK),
                            bv_bc[:, nh * VB : nh * VB + VB].rearrange(
                                "p (h e) -> p h e", e=DK
                            ),
                            mybir.AluOpType.add,
                        )

            # ================= Phase 2: attention =================
            with tc.tile_pool(name="mpool", bufs=1) as mpool, \
                 tc.tile_pool(name="empool", bufs=2) as empool, \
                 tc.tile_pool(name="stpool", bufs=2) as stpool, \
                 tc.tile_pool(name="smpool", bufs=3) as smpool, \
                 tc.tile_pool(name="spsum", bufs=2, space="PSUM") as spsum, \
                 tc.tile_pool(name="cpsum", bufs=1, space="PSUM") as cpsum, \
                 tc.tile_pool(name="tpsum", bufs=2, space="PSUM") as tpsum, \
                 tc.tile_pool(name="rpsum", bufs=1, space="PSUM") as rpsum:
                cnpsum = rpsum

                maskT_sb = mpool.tile([P, KC, SQ], BF16)
                nc.sync.dma_start(
                    maskT_sb, maskT.ap().rearrange("(c p) q -> p c q", p=P)
                )

                for qb in range(NQB):
                    q0 = qb * QBLK
                    for h in range(H):
                        pr0 = (h % HPC) * DK     # partition offset within d-chunk
                        dch = h // HPC           # which d-chunk holds this head
                        em = empool.tile([P, KC, QBLK], BF16, tag="em")
                        pc = cpsum.tile([DK + 1, QBLK], F32, tag="pc")
                        KP = 2 if KC % 2 == 0 else 1   # kc per scores psum
                        for kc0 in range(0, KC, KP):
                            ps = spsum.tile([P, KP * QBLK], F32, tag="ps")
                            for kk in range(KP):
                                kc = kc0 + kk
                                nc.tensor.matmul(
                                    ps[:, kk * QBLK : (kk + 1) * QBLK],
                                    lhsT=kT_sb[pr0 : pr0 + DK, dch, kc * P : kc * P + P],
                                    rhs=qT_sb[pr0 : pr0 + DK, dch, q0 : q0 + QBLK],
                                )
                            nc.scalar.activation(
                                em[:, kc0 : kc0 + KP, :], ps, AF.Exp,
                                scale=1.0 / np.sqrt(DK),
                            )
                            m_sl = maskT_sb[:, kc0 : kc0 + KP, q0 : q0 + QBLK]
                            e_sl = em[:, kc0 : kc0 + KP, :]
                            if (kc0 // KP) % 3 == 0:
                                nc.gpsimd.tensor_mul(e_sl, e_sl, m_sl)
                            else:
                                nc.vector.tensor_mul(e_sl, e_sl, m_sl)
                            for kk in range(KP):
                                kc = kc0 + kk
                                nc.tensor.matmul(
                                    pc,
                                    lhsT=vaug_sb[:, kc, h * (DK + 1) : (h + 1) * (DK + 1)],
                                    rhs=em[:, kc, :],
                                    start=(kc == 0),
                                    stop=(kc == KC - 1),
                                )
                        # softmax denominators -> per-q-column reciprocals
                        srow = smpool.tile([1, QBLK], F32, tag="srow")
                        nc.scalar.copy(srow, pc[DK : DK + 1, :])
                        prc = rpsum.tile([P, QCC], F32, tag="rp", name="prc")
                        for j in range(QCC):
                            nc.tensor.matmul(
                                prc[:, j : j + 1],
                                lhsT=srow[0:1, j * P : j * P + P],
                                rhs=ones11,
                            )
                        recip = smpool.tile([P, QCC], F32, tag="recip")
                        nc.vector.reciprocal(recip, prc)

                        # context rows: transpose + normalize -> c_nat
                        crow = smpool.tile([DK, QBLK], BF16, tag="crow")
                        nc.scalar.copy(crow, pc[0:DK, :])
                        pcn = cnpsum.tile([P, QCC * DK], BF16, tag="rp", name="pcn")
                        for j in range(QCC):
                            nc.tensor.transpose(
                                pcn[:, j * DK : (j + 1) * DK],
                                crow[:, j * P : j * P + P],
                                id_bf[:DK, :DK],
                            )
                        for j in range(QCC):
                            nc.scalar.activation(
                                c_nat[:, qb * QCC + j, h * DK : (h + 1) * DK],
                                pcn[:, j * DK : (j + 1) * DK],
                                AF.Copy,
                                scale=recip[:, j : j + 1],
                            )

                        # attention probabilities: transpose, normalize, DMA out
                        for j in range(QCC):
                            stage = stpool.tile([P, SK], F32, tag="stage")
                            for kg in range(NKG):
                                pt = tpsum.tile([P, KGRP * P], BF16, tag="pt")
                                for kk in range(KGRP):
                                    kc = kg * KGRP + kk
                                    nc.tensor.transpose(
                                        pt[:, kk * P : kk * P + P],
                                        em[:, kc, j * P : j * P + P],
                                        id_bf,
                                    )
                                dst = stage[:, kg * KGRP * P : (kg + 1) * KGRP * P]
                                if kg % 2 == 0:
                                    nc.scalar.activation(
                                        dst, pt, AF.Copy, scale=recip[:, j : j + 1]
                                    )
                                else:
                                    nc.vector.tensor_scalar_mul(
                                        dst, pt, recip[:, j : j + 1]
                                    )
                            nc.sync.dma_start(
                                attn_out.ap()[h, (qb * QCC + j) * P : (qb * QCC + j + 1) * P, :],
                                stage,
                            )

            # ================= Phase 3: out proj + residual + LN =================
            with tc.tile_pool(name="wopool", bufs=1) as wopool, \
                 tc.tile_pool(name="bcpool", bufs=1) as bcpool, \
                 tc.tile_pool(name="opool", bufs=2) as opool, \
                 tc.tile_pool(name="lnpool", bufs=2) as lnpool, \
                 tc.tile_pool(name="p3psum", bufs=2, space="PSUM") as p3psum, \
                 tc.tile_pool(name="ctpsum", bufs=2, space="PSUM") as ctpsum:

                # cT: [d part, q free] from c_nat via PE transposes
                cT_sb = wopool.tile([P, DCH, SQ], BF16, tag="cT")
                DG = 4 if DCH % 4 == 0 else DCH
                for qn in range(QN):
                    for dcg in range(DCH // DG):
                        pw = ctpsum.tile([P, DG * P], BF16, tag="pw")
                        for dd in range(DG):
                            dc = dcg * DG + dd
                            nc.tensor.transpose(
                                pw[:, dd * P : dd * P + P],
                                c_nat[:, qn, dc * P : dc * P + P],
                                id_bf,
                            )
                        nc.vector.tensor_copy(
                            cT_sb[:, dcg * DG : dcg * DG + DG, qn * P : qn * P + P],
                            pw.rearrange("p (c q) -> p c q", q=P),
                        )

                wo_bf = wopool.tile([P, DCH, D], BF16, tag="wobf")
                nc.sync.dma_start(wo_bf, woT.ap().rearrange("(c p) n -> p c n", p=P))

                bo_bc = wopool.tile([P, D], F32, tag="bo")
                nc.sync.dma_start(bo_bc, bcast(bo.ap()))
                g_bc = wopool.tile([P, D], F32, tag="g")
                nc.sync.dma_start(g_bc, bcast(ln_g.ap()))
                b_bc = wopool.tile([P, D], F32, tag="b")
                nc.sync.dma_start(b_bc, bcast(ln_b.ap()))

                for qn in range(QN):
                    onat = opool.tile([P, D], F32, tag="onat")
                    for nh in range(D // VB):
                        po = p3psum.tile([P, VB], F32, tag="po")
                        for dci in range(DCH):
                            nc.tensor.matmul(
                                po,
                                lhsT=cT_sb[:, dci, qn * P : qn * P + P],
                                rhs=wo_bf[:, dci, nh * VB : nh * VB + VB],
                                start=(dci == 0),
                                stop=(dci == DCH - 1),
                            )
                        nc.vector.tensor_tensor(
                            onat[:, nh * VB : nh * VB + VB],
                            po,
                            bo_bc[:, nh * VB : nh * VB + VB],
                            mybir.AluOpType.add,
                        )
                    xr = opool.tile([P, D], F32, tag="xr")
                    nc.sync.dma_start(xr, xq_res.ap()[qn * P : qn * P + P, :])
                    nc.vector.tensor_add(onat, onat, xr)

                    # LayerNorm
                    nmean = lnpool.tile([P, 1], F32, tag="nmean")
                    nc.vector.reduce_sum(nmean, onat, axis=mybir.AxisListType.X)
                    nc.vector.tensor_scalar_mul(nmean, nmean, -1.0 / D)
                    xc = lnpool.tile([P, D], F32, tag="xc")
                    nc.vector.tensor_scalar_add(xc, onat, nmean[:, 0:1])
                    vs = lnpool.tile([P, 1], F32, tag="vs")
                    nc.scalar.activation(onat, xc, AF.Square, accum_out=vs)
                    nc.vector.tensor_scalar_mul(vs, vs, 1.0 / D)
                    nc.vector.tensor_scalar_add(vs, vs, 1e-5)
                    st = lnpool.tile([P, 1], F32, tag="st")
                    nc.scalar.activation(st, vs, AF.Sqrt)
                    rstd = lnpool.tile([P, 1], F32, tag="rstd")
                    nc.vector.reciprocal(rstd, st)
                    nc.vector.tensor_scalar_mul(xc, xc, rstd[:, 0:1])
                    nc.vector.tensor_mul(xc, xc, g_bc)
                    nc.vector.tensor_add(xc, xc, b_bc)
                    nc.sync.dma_start(y_out.ap()[qn * P : qn * P + P, :], xc)

    nc.compile()
    return nc


def _get_program(SQ, SK, D, H):
    key = (SQ, SK, D, H)
    if key not in _PROGRAM_CACHE:
        _PROGRAM_CACHE[key] = _build_program(SQ, SK, D, H)
    return _PROGRAM_CACHE[key]


def _make_in_maps(Q, K, V, attn_mask, Wq, bq, Wk, bk, Wv, bv, Wo, bo, ln_g, ln_b,
                  n_cores=8):
    f32 = np.float32
    bf16 = ml_dtypes.bfloat16
    SQ = Q.shape[1] * Q.shape[0] // n_cores  # rows per core
    per_batch = n_cores // Q.shape[0]

    wqT = np.ascontiguousarray(np.asarray(Wq, f32).T).astype(bf16)
    wkT = np.ascontiguousarray(np.asarray(Wk, f32).T).astype(bf16)
    wvT = np.ascontiguousarray(np.asarray(Wv, f32).T).astype(bf16)
    woT = np.ascontiguousarray(np.asarray(Wo, f32).T).astype(bf16)
    shared = {
        "wqT": wqT, "wkT": wkT, "wvT": wvT, "woT": woT,
        "bq": np.asarray(bq, f32), "bk": np.asarray(bk, f32),
        "bv": np.asarray(bv, f32), "bo": np.asarray(bo, f32),
        "ln_g": np.asarray(ln_g, f32), "ln_b": np.asarray(ln_b, f32),
    }

    in_maps = []
    kT_cache = {}
    for c in range(n_cores):
        b, sh = c // per_batch, c % per_batch
        qsl = np.asarray(Q[b, sh * SQ : (sh + 1) * SQ], f32)
        if b not in kT_cache:
            kT_cache[b] = (
                np.ascontiguousarray(np.asarray(K[b], f32).T).astype(bf16),
                np.ascontiguousarray(np.asarray(V[b], f32).T).astype(bf16),
            )
        xkT, xvT = kT_cache[b]
        m = np.asarray(attn_mask[b, sh * SQ : (sh + 1) * SQ])  # [SQ, SK] int
        maskT = np.ascontiguousarray(m.T).astype(bf16)
        in_maps.append({
            "xqT": np.ascontiguousarray(qsl.T).astype(bf16),
            "xkT": xkT,
            "xvT": xvT,
            "xq_res": qsl,
            "maskT": maskT,
            **shared,
        })
    return in_maps


def kernel(Q, K, V, attn_mask, Wq, bq, Wk, bk, Wv, bv, Wo, bo, ln_g, ln_b):
    from concourse.bass_utils import run_bass_kernel_spmd

    Q = np.asarray(Q)
    Bsz, Ssz, Dsz = Q.shape
    n_cores = 8
    per_batch = n_cores // Bsz
    SQ = Ssz // per_batch
    H = N_HEADS

    nc = _get_program(SQ, Ssz, Dsz, H)
    in_maps = _make_in_maps(Q, K, V, attn_mask, Wq, bq, Wk, bk, Wv, bv,
                            Wo, bo, ln_g, ln_b, n_cores)
    res = run_bass_kernel_spmd(nc, in_maps, list(range(n_cores)))

    y_full = np.empty((Bsz, Ssz, Dsz), np.float32)
    attn_full = np.empty((Bsz, H, Ssz, Ssz), np.float32)
    for c, r in enumerate(res.results):
        b, sh = c // per_batch, c % per_batch
        y_full[b, sh * SQ : (sh + 1) * SQ] = r["y"]
        attn_full[b, :, sh * SQ : (sh + 1) * SQ, :] = r["attn"]
    return (y_full, attn_full)
